# revision 7
# baseline (speedup 1.0000x reference)
"""Weighted-MSE loss (Euler-angle + attribute weights) on 8 trn2 NeuronCores.

loss = mean(weight * (inp - label)^2),
  weight[i] = (sum_j 1-cos(ea[i,j])) * (sum_c attribute[i,c] * inv_freq[c])

Pure data-parallel over the batch dim; each of the 8 cores gets 4096 rows
(32 segments of 512 columns per SBUF partition). inp/label ship as fp8
e4m3 (2 MiB each per core, ~1.1e-3 end-to-end error, validated on
host + hardware). label is negated on host so every subtract is an ADD.

Hard-won scheduling facts baked into this version:
- HWDGE DMAs are ISSUED by the Sync or ACT engine; putting DMAs on the
  scalar ring serializes them with ACT squares (v3/v4 regression). All
  DMAs live on the sync ring here; the Sync engine does nothing else.
- The CCE accumulate-DMA costs ~4x SDMA engine-time per byte and starves
  concurrent plain traffic via the per-engine round-robin, so it is not
  used at all.
- Subtract capacity is the wall; it is split across DVE tensor_add
  (0.96 elems/partition/ns) and GpSimd tensor_add (software Q7 impl,
  ~0.5): GpSimd takes 12 of 32 segments, DVE 20, all writing fp16 diffs.
- Squares: ACT (1.2, always 1x) takes 24 segments, DVE squares 8 of its
  own early segments at fp16 2x mode.

The per-row weighted reduction runs on TensorE: 32 matmuls of
psum[1,512] += w[:,n].T @ sq[:, n*512:(n+1)*512] (per-row weight in the
[128,1] fp16 stationary, PSUM accumulates), one DVE reduce of [1,512]
at the end. Weight DMAs (fp16 ea/attr/invf) go first on the sync ring;
Sin is ACT's first op so one activation-table load (trig_and_small)
covers both Sin and Square.
"""

import math

import numpy as np

B, D = 32768, 512
M = 8  # cores
BS = B // M  # 4096 rows per core
P = 128  # SBUF partitions
NSEG = BS // P  # 32 row-segments of 512 per partition
NATTR = 6
GRP = 4  # segs per subtract/square group
NGRP = NSEG // GRP  # 8 groups
PIECES = [4, 4, 8, 8, 8]  # DMA piece sizes in segments
GPS_GROUPS = (0, 2, 5)  # subtract groups on GpSimd (rest on DVE)
DVE_SQ_GROUPS = (1, 3)  # square groups on DVE (rest on ACT)

_cache: dict = {}


def _build():
    import concourse.bacc as bacc
    import concourse.mybir as mybir
    import concourse.tile as tile

    nc = bacc.Bacc(
        "TRN2",
        debug=False,
        enable_asserts=False,
        num_devices=M,
    )
    f32 = mybir.dt.float32
    f16 = mybir.dt.float16
    f8 = mybir.dt.float8e4

    inp = nc.dram_tensor("inp", [BS, D], f8, kind="ExternalInput").ap()
    lab = nc.dram_tensor("label", [BS, D], f8, kind="ExternalInput").ap()
    ea = nc.dram_tensor("ea", [BS, 3], f16, kind="ExternalInput").ap()
    attr = nc.dram_tensor("attr", [BS, NATTR], f16, kind="ExternalInput").ap()
    invf = nc.dram_tensor("invf", [P, NSEG * NATTR], f16, kind="ExternalInput").ap()
    out = nc.dram_tensor("out", [1, 1], f32, kind="ExternalOutput").ap()

    inp_v = inp.rearrange("(p n) d -> p n d", p=P)  # [128, 32, 512]
    lab_v = lab.rearrange("(p n) d -> p n d", p=P)
    ea_v = ea.rearrange("(p n) t -> p n t", p=P)
    attr_v = attr.rearrange("(p n) c -> p n c", p=P)

    ADD = mybir.AluOpType.add
    MULT = mybir.AluOpType.mult
    AXX = mybir.AxisListType.X

    with tile.TileContext(nc) as tc:
        with (
            tc.tile_pool(name="big", bufs=1) as big,
            tc.tile_pool(name="small", bufs=1) as small,
            tc.tile_pool(name="psum", bufs=1, space="PSUM") as psum,
        ):
            inp8 = big.tile([P, NSEG * D], f8)
            lab8 = big.tile([P, NSEG * D], f8)
            diff16 = big.tile([P, NSEG * D], f16)
            sq_t = big.tile([P, NSEG * D], f16)
            acc = psum.tile([1, D], f32)

            def seg3(t, s0, n):  # [P, n, D] view of segs s0..s0+n of tile t
                return t[:, s0 * D : (s0 + n) * D].rearrange(
                    "p (n d) -> p n d", d=D
                )

            # ---- sync ring: weights first, then inp/lab piece pairs ----
            ea_t = small.tile([P, NSEG * 3], f16)
            nc.sync.dma_start(ea_t[:].rearrange("p (n t) -> p n t", t=3), ea_v)
            attr_t = small.tile([P, NSEG * NATTR], f16)
            nc.sync.dma_start(
                attr_t[:].rearrange("p (n c) -> p n c", c=NATTR), attr_v
            )
            invf_t = small.tile([P, NSEG * NATTR], f16)
            nc.sync.dma_start(invf_t[:], invf)
            s0 = 0
            for npcs in PIECES:
                nc.sync.dma_start(seg3(inp8, s0, npcs), inp_v[:, s0 : s0 + npcs, :])
                nc.sync.dma_start(seg3(lab8, s0, npcs), lab_v[:, s0 : s0 + npcs, :])
                s0 += npcs

            # ---- weights (Sin is ACT's first op) ----
            half = small.tile([P, NSEG * 3], f16)
            nc.vector.tensor_scalar(
                half[:], ea_t[:], 0.5, math.pi, MULT, mybir.AluOpType.min
            )
            nc.vector.tensor_scalar_max(half[:], half[:], -math.pi)
            sin_t = small.tile([P, NSEG * 3], f16)
            nc.scalar.activation(
                sin_t[:], half[:], mybir.ActivationFunctionType.Sin
            )
            sin2 = small.tile([P, NSEG * 3], f16)
            nc.vector.tensor_mul(sin2[:], sin_t[:], sin_t[:])
            csum = small.tile([P, NSEG], f32)
            nc.vector.tensor_reduce(
                csum[:], sin2[:].rearrange("p (n t) -> p n t", t=3), axis=AXX, op=ADD
            )
            awe = small.tile([P, NSEG * NATTR], f16)
            nc.vector.tensor_mul(awe[:], attr_t[:], invf_t[:])
            attrw = small.tile([P, NSEG], f32)
            nc.vector.tensor_reduce(
                attrw[:],
                awe[:].rearrange("p (n c) -> p n c", c=NATTR),
                axis=AXX,
                op=ADD,
            )
            w16 = small.tile([P, NSEG], f16)
            nc.vector.tensor_mul(w16[:], csum[:], attrw[:])

            mm = [0]

            def matmuls(g):
                for n_ in range(g * GRP, (g + 1) * GRP):
                    nc.tensor.matmul(
                        acc[:],
                        w16[:, n_ : n_ + 1],
                        sq_t[:, n_ * D : (n_ + 1) * D],
                        start=(mm[0] == 0),
                        stop=(mm[0] == NSEG - 1),
                    )
                    mm[0] += 1

            # ---- subtract + square + matmul per 4-seg group ----
            for g in range(NGRP):
                a = g * GRP * D
                b = (g + 1) * GRP * D
                eng = nc.gpsimd if g in GPS_GROUPS else nc.vector
                eng.tensor_add(diff16[:, a:b], inp8[:, a:b], lab8[:, a:b])
                if g in DVE_SQ_GROUPS:
                    nc.vector.tensor_mul(
                        sq_t[:, a:b], diff16[:, a:b], diff16[:, a:b]
                    )
                else:
                    nc.scalar.activation(
                        sq_t[:, a:b],
                        diff16[:, a:b],
                        mybir.ActivationFunctionType.Square,
                    )
                matmuls(g)
            assert mm[0] == NSEG

            # ---- epilogue ----
            part = small.tile([1, 1], f32)
            nc.vector.tensor_reduce(part[:], acc[:], axis=AXX, op=ADD)
            nc.sync.dma_start(out, part[:])

    nc.compile()
    return nc


def get_nc():
    if "nc" not in _cache:
        _cache["nc"] = _build()
    return _cache["nc"]


def make_in_maps(inp, label, ea, attribute, attribute_num):
    import ml_dtypes

    f8 = ml_dtypes.float8_e4m3
    inv_freq2 = (
        2.0
        * np.asarray(attribute_num, dtype=np.float64).sum()
        / np.asarray(attribute_num, dtype=np.float64)
    ).astype(np.float16)
    invf_tiled = np.ascontiguousarray(
        np.broadcast_to(np.tile(inv_freq2, NSEG), (P, NSEG * NATTR))
    )
    inp8 = np.asarray(inp, dtype=f8)
    lab8 = (-np.asarray(label, dtype=np.float32)).astype(f8)
    ea16 = np.asarray(ea, dtype=np.float16)
    attr16 = np.asarray(attribute, dtype=np.float16)
    in_maps = []
    for c in range(M):
        s = slice(c * BS, (c + 1) * BS)
        in_maps.append(
            {
                "inp": np.ascontiguousarray(inp8[s]),
                "label": np.ascontiguousarray(lab8[s]),
                "ea": np.ascontiguousarray(ea16[s]),
                "attr": np.ascontiguousarray(attr16[s]),
                "invf": invf_tiled,
            }
        )
    return in_maps


def kernel(inp, label, ea, attribute, attribute_num, batch_size=None, **_ignored):
    from concourse import bass_utils

    nc = get_nc()
    in_maps = make_in_maps(
        np.asarray(inp, dtype=np.float32),
        np.asarray(label, dtype=np.float32),
        np.asarray(ea, dtype=np.float32),
        np.asarray(attribute, dtype=np.int32),
        np.asarray(attribute_num, dtype=np.float32),
    )
    res = bass_utils.run_bass_kernel_spmd(nc, in_maps, core_ids=list(range(M)))
    total = 0.0
    for r in res.results:
        total += float(np.asarray(r["out"], dtype=np.float64)[0, 0])
    return np.float32(total / (B * D))


# revision 8
# speedup vs baseline: 1.1742x; 1.1742x over previous
"""Weighted-MSE loss (Euler-angle + attribute weights) on 8 trn2 NeuronCores.

loss = mean(weight * (inp - label)^2),
  weight[i] = (sum_j 1-cos(ea[i,j])) * (sum_c attribute[i,c] * inv_freq[c])

Pure data-parallel over the batch dim; each of the 8 cores gets 4096 rows
(32 segments of 512 columns per SBUF partition). label is negated on host
so every subtract is an ADD. Measured-on-hardware facts that shape this
version:

- All DMAs ride the sync ring (~390 GB/s aggregate). DMAs issued by the
  scalar ring serialize with ACT compute; the CCE accumulate-DMA and
  GpSimd tensor ops both proved net losses (fabric tax / DVE interference).
- DVE tensor_tensor runs 2x only with all-2-byte operands, so 12 of 32
  segments ship as fp16 (2x sub) and 20 as fp8 (1x sub); everything else
  is fp8 to keep HBM bytes down (5 MiB/core total). Quantization error
  ~1e-3, validated on host + hardware.
- Squares: ACT takes 28 segments (1 elem/cycle, one trig_and_small table
  load shared with Sin), DVE squares the last fp8 group from its fp16
  diffs at 2x to shorten the tail.
- The per-row weighted reduction is 32 TensorE matmuls of
  psum[1,512] += w[:,n].T @ sq[:, n*512:(n+1)*512] (per-row weight in
  the [128,1] fp16 stationary, PSUM accumulates) + one DVE reduce of
  [1,512]. A burst of dummy matmuls runs during the DMA phase to climb
  the PE p-state ramp so the real matmuls run near 213ns instead of
  ~630ns.
"""

import math

import numpy as np

B, D = 32768, 512
M = 8  # cores
BS = B // M  # 4096 rows per core
P = 128  # SBUF partitions
NSEG = BS // P  # 32 row-segments of 512 per partition
NATTR = 6
GRP = 4  # segs per subtract/square group
F16SEG = 12  # segs 0..11 ship fp16; 12..31 ship fp8
PIECES16 = [8, 4]  # fp16 DMA piece sizes (segs)
PIECES8 = [8, 8, 4]  # fp8 DMA piece sizes (segs)
DVE_SQ_GROUPS = (7,)  # square groups on DVE (rest on ACT)
N_WARMUP_MM = 14  # dummy matmuls to climb the PE p-state ramp

_cache: dict = {}


def _build():
    import concourse.bacc as bacc
    import concourse.mybir as mybir
    import concourse.tile as tile

    nc = bacc.Bacc(
        "TRN2",
        debug=False,
        enable_asserts=False,
        num_devices=M,
    )
    f32 = mybir.dt.float32
    f16 = mybir.dt.float16
    f8 = mybir.dt.float8e4

    n16 = P * F16SEG  # rows shipped as fp16
    n8 = P * (NSEG - F16SEG)
    inp16 = nc.dram_tensor("inp16", [n16, D], f16, kind="ExternalInput").ap()
    lab16 = nc.dram_tensor("lab16", [n16, D], f16, kind="ExternalInput").ap()
    inp8 = nc.dram_tensor("inp8", [n8, D], f8, kind="ExternalInput").ap()
    lab8 = nc.dram_tensor("lab8", [n8, D], f8, kind="ExternalInput").ap()
    ea = nc.dram_tensor("ea", [BS, 3], f16, kind="ExternalInput").ap()
    attr = nc.dram_tensor("attr", [BS, NATTR], f16, kind="ExternalInput").ap()
    invf = nc.dram_tensor("invf", [P, NSEG * NATTR], f16, kind="ExternalInput").ap()
    out = nc.dram_tensor("out", [1, 1], f32, kind="ExternalOutput").ap()

    # partition p <-> original rows p*32..p*32+31; host pre-splits rows
    # n<F16SEG into the fp16 tensors so each is (p n) d contiguous.
    i16_v = inp16.rearrange("(p n) d -> p n d", p=P)  # [128, 12, 512]
    l16_v = lab16.rearrange("(p n) d -> p n d", p=P)
    i8_v = inp8.rearrange("(p n) d -> p n d", p=P)  # [128, 20, 512]
    l8_v = lab8.rearrange("(p n) d -> p n d", p=P)
    ea_v = ea.rearrange("(p n) t -> p n t", p=P)
    attr_v = attr.rearrange("(p n) c -> p n c", p=P)

    ADD = mybir.AluOpType.add
    MULT = mybir.AluOpType.mult
    AXX = mybir.AxisListType.X

    with tile.TileContext(nc) as tc:
        with (
            tc.tile_pool(name="big", bufs=1) as big,
            tc.tile_pool(name="small", bufs=1) as small,
            tc.tile_pool(name="psum", bufs=1, space="PSUM") as psum,
        ):
            in16_t = big.tile([P, F16SEG * D], f16)
            la16_t = big.tile([P, F16SEG * D], f16)
            in8_t = big.tile([P, (NSEG - F16SEG) * D], f8)
            la8_t = big.tile([P, (NSEG - F16SEG) * D], f8)
            diff16 = big.tile([P, NSEG * D], f16)
            sq_t = big.tile([P, NSEG * D], f16)
            acc = psum.tile([1, D], f32)
            junk = psum.tile([1, D], f32)
            scr16 = small.tile([P, D], f16)
            nc.gpsimd.memset(scr16[:], 1.0)

            def seg3(t, s0, n):
                return t[:, s0 * D : (s0 + n) * D].rearrange(
                    "p (n d) -> p n d", d=D
                )

            # ---- sync ring: fp16 pair 0, weights, rest of the pieces ----
            s = 0
            for k, npcs in enumerate(PIECES16):
                nc.sync.dma_start(seg3(in16_t, s, npcs), i16_v[:, s : s + npcs, :])
                nc.sync.dma_start(seg3(la16_t, s, npcs), l16_v[:, s : s + npcs, :])
                if k == 0:
                    ea_t = small.tile([P, NSEG * 3], f16)
                    nc.sync.dma_start(
                        ea_t[:].rearrange("p (n t) -> p n t", t=3), ea_v
                    )
                    attr_t = small.tile([P, NSEG * NATTR], f16)
                    nc.sync.dma_start(
                        attr_t[:].rearrange("p (n c) -> p n c", c=NATTR), attr_v
                    )
                    invf_t = small.tile([P, NSEG * NATTR], f16)
                    nc.sync.dma_start(invf_t[:], invf)
                s += npcs
            s = 0
            for npcs in PIECES8:
                nc.sync.dma_start(seg3(in8_t, s, npcs), i8_v[:, s : s + npcs, :])
                nc.sync.dma_start(seg3(la8_t, s, npcs), l8_v[:, s : s + npcs, :])
                s += npcs

            # ---- PE p-state warmup: dummy matmuls, no data deps ----
            for _ in range(N_WARMUP_MM):
                nc.tensor.matmul(
                    junk[:], scr16[:, 0:1], scr16[:], start=True, stop=True
                )

            # ---- weights ----
            half = small.tile([P, NSEG * 3], f16)
            nc.vector.tensor_scalar(
                half[:], ea_t[:], 0.5, math.pi, MULT, mybir.AluOpType.min
            )
            nc.vector.tensor_scalar_max(half[:], half[:], -math.pi)
            sin_t = small.tile([P, NSEG * 3], f16)
            nc.scalar.activation(
                sin_t[:], half[:], mybir.ActivationFunctionType.Sin
            )
            sin2 = small.tile([P, NSEG * 3], f16)
            nc.vector.tensor_mul(sin2[:], sin_t[:], sin_t[:])
            csum = small.tile([P, NSEG], f32)
            nc.vector.tensor_reduce(
                csum[:], sin2[:].rearrange("p (n t) -> p n t", t=3), axis=AXX, op=ADD
            )
            awe = small.tile([P, NSEG * NATTR], f16)
            nc.vector.tensor_mul(awe[:], attr_t[:], invf_t[:])
            attrw = small.tile([P, NSEG], f32)
            nc.vector.tensor_reduce(
                attrw[:],
                awe[:].rearrange("p (n c) -> p n c", c=NATTR),
                axis=AXX,
                op=ADD,
            )
            w16 = small.tile([P, NSEG], f16)
            nc.vector.tensor_mul(w16[:], csum[:], attrw[:])

            mm = [0]

            def matmuls(g):
                for n_ in range(g * GRP, (g + 1) * GRP):
                    nc.tensor.matmul(
                        acc[:],
                        w16[:, n_ : n_ + 1],
                        sq_t[:, n_ * D : (n_ + 1) * D],
                        start=(mm[0] == 0),
                        stop=(mm[0] == NSEG - 1),
                    )
                    mm[0] += 1

            # ---- subtract + square + matmul per 4-seg group ----
            for g in range(NSEG // GRP):
                a = g * GRP * D
                b = (g + 1) * GRP * D
                if g * GRP < F16SEG:  # fp16 region: DVE sub at 2x
                    nc.vector.tensor_add(
                        diff16[:, a:b], in16_t[:, a:b], la16_t[:, a:b]
                    )
                else:  # fp8 region: DVE sub at 1x
                    a8 = a - F16SEG * D
                    b8 = b - F16SEG * D
                    nc.vector.tensor_add(
                        diff16[:, a:b], in8_t[:, a8:b8], la8_t[:, a8:b8]
                    )
                if g in DVE_SQ_GROUPS:
                    nc.vector.tensor_mul(
                        sq_t[:, a:b], diff16[:, a:b], diff16[:, a:b]
                    )
                else:
                    nc.scalar.activation(
                        sq_t[:, a:b],
                        diff16[:, a:b],
                        mybir.ActivationFunctionType.Square,
                    )
                matmuls(g)
            assert mm[0] == NSEG

            # ---- epilogue ----
            part = small.tile([1, 1], f32)
            nc.vector.tensor_reduce(part[:], acc[:], axis=AXX, op=ADD)
            nc.sync.dma_start(out, part[:])

    nc.compile()
    return nc


def get_nc():
    if "nc" not in _cache:
        _cache["nc"] = _build()
    return _cache["nc"]


def make_in_maps(inp, label, ea, attribute, attribute_num):
    import ml_dtypes

    f8 = ml_dtypes.float8_e4m3
    inv_freq2 = (
        2.0
        * np.asarray(attribute_num, dtype=np.float64).sum()
        / np.asarray(attribute_num, dtype=np.float64)
    ).astype(np.float16)
    invf_tiled = np.ascontiguousarray(
        np.broadcast_to(np.tile(inv_freq2, NSEG), (P, NSEG * NATTR))
    )
    inp32 = np.asarray(inp, dtype=np.float32)
    lab32 = -np.asarray(label, dtype=np.float32)
    ea16 = np.asarray(ea, dtype=np.float16)
    attr16 = np.asarray(attribute, dtype=np.float16)
    in_maps = []
    for c in range(M):
        s = slice(c * BS, (c + 1) * BS)
        iv = inp32[s].reshape(P, NSEG, D)
        lv = lab32[s].reshape(P, NSEG, D)
        in_maps.append(
            {
                "inp16": np.ascontiguousarray(
                    iv[:, :F16SEG].reshape(-1, D).astype(np.float16)
                ),
                "lab16": np.ascontiguousarray(
                    lv[:, :F16SEG].reshape(-1, D).astype(np.float16)
                ),
                "inp8": np.ascontiguousarray(
                    iv[:, F16SEG:].reshape(-1, D).astype(f8)
                ),
                "lab8": np.ascontiguousarray(
                    lv[:, F16SEG:].reshape(-1, D).astype(f8)
                ),
                "ea": np.ascontiguousarray(ea16[s]),
                "attr": np.ascontiguousarray(attr16[s]),
                "invf": invf_tiled,
            }
        )
    return in_maps


def kernel(inp, label, ea, attribute, attribute_num, batch_size=None, **_ignored):
    from concourse import bass_utils

    nc = get_nc()
    in_maps = make_in_maps(
        np.asarray(inp, dtype=np.float32),
        np.asarray(label, dtype=np.float32),
        np.asarray(ea, dtype=np.float32),
        np.asarray(attribute, dtype=np.int32),
        np.asarray(attribute_num, dtype=np.float32),
    )
    res = bass_utils.run_bass_kernel_spmd(nc, in_maps, core_ids=list(range(M)))
    total = 0.0
    for r in res.results:
        total += float(np.asarray(r["out"], dtype=np.float64)[0, 0])
    return np.float32(total / (B * D))
